# revision 1
# baseline (speedup 1.0000x reference)
"""Trainium2 Bass kernel for nn_Critic (han 1008->2048->2048->512, q-MLP 520->2048->2048->1).

Data-parallel over 8 NeuronCores: batch 8192 -> 1024 rows/core, weights replicated.
Activations live in SBUF feature-major (x^T: [features, batch]); every layer is
out^T[f_tile, b_tile] += W[k_block, f_block].T @ x^T[k_block, b_tile], so weight
blocks are consumed in their natural DRAM layout and layer outputs are already
feature-major for the next layer. Matmuls run as float32r (FP22 multiply, fp32
accumulate) for full PE rate; bias+ReLU fuse into one ScalarE activation per tile
on the PSUM->SBUF path.
"""

import sys

sys.path.insert(0, "/opt/trn_rl_repo")

import numpy as np

N_CORES = 8
BATCH = 8192
B = BATCH // N_CORES  # rows per core
BT = 512              # batch tile = psum free dim
NB = B // BT          # batch tiles per core
FG = 2                # feature (output) 128-tiles per psum group; FG*NB banks/group

OBS_DIM = 1008
HAN_HIDDEN = 2048
HAN_OUT = 512
ACTION_DIM = 8
MLP_HIDDEN = 2048

F32 = None  # set in _build (mybir types)


def _k_tiles(K):
    """Split contraction dim K into 128-partition tiles."""
    sizes = []
    while K > 0:
        sizes.append(min(128, K))
        K -= 128
    return sizes


def _split_excess_waits(nc, max_waits=1):
    """Walrus codegen rejects instructions carrying more than ~1 embedded sync
    wait (notably fused 4-byte matmuls and NO-type control instructions).
    Move overflow waits onto same-engine single-wait NoOps inserted just
    before the instruction — the engine queue is in-order, so semantics are
    identical."""
    import concourse.mybir as mybir

    ctr = 0
    for func in nc.m.functions:
        for blk in func.blocks:
            out = []
            for inst in blk.instructions:
                si = inst.sync_info
                if si is not None and len(si.on_wait) > max_waits:
                    waits = list(si.on_wait)
                    while len(waits) > max_waits:
                        nop = mybir.InstNoOp(
                            name=f"waitsplit_nop_{ctr}", ins=[], outs=[])
                        ctr += 1
                        nop.engine = inst.engine
                        nop.bass_nofuse = True
                        nop.sync_info = mybir.SyncInfo(
                            on_wait=[waits.pop(0)], on_update=[])
                        nc.register_instruction(nop, overwrite=True)
                        out.append(nop)
                    inst.sync_info = mybir.SyncInfo(
                        on_wait=waits, on_update=list(si.on_update))
                out.append(inst)
            blk.instructions[:] = out
    return ctr


def _build(repeats=1, mode='full'):
    import concourse.bass as bass
    import concourse.mybir as mybir
    import concourse.tile as tile

    f32 = mybir.dt.float32
    f32r = mybir.dt.float32r
    Relu = mybir.ActivationFunctionType.Relu
    Ident = mybir.ActivationFunctionType.Identity

    nc = bass.Bass()

    # --- DRAM I/O (per-core shard shapes) ---
    obsT = nc.dram_tensor("obsT", [OBS_DIM, B], f32r, kind="ExternalInput")
    actT = nc.dram_tensor("actT", [ACTION_DIM, B], f32r, kind="ExternalInput")
    W1 = nc.dram_tensor("W1", [OBS_DIM, HAN_HIDDEN], f32r, kind="ExternalInput")
    W2 = nc.dram_tensor("W2", [HAN_HIDDEN, HAN_HIDDEN], f32r, kind="ExternalInput")
    W3 = nc.dram_tensor("W3", [HAN_HIDDEN, HAN_OUT], f32r, kind="ExternalInput")
    Wm1a = nc.dram_tensor("Wm1a", [ACTION_DIM, MLP_HIDDEN], f32r, kind="ExternalInput")
    Wm1e = nc.dram_tensor("Wm1e", [HAN_OUT, MLP_HIDDEN], f32r, kind="ExternalInput")
    Wm2 = nc.dram_tensor("Wm2", [MLP_HIDDEN, MLP_HIDDEN], f32r, kind="ExternalInput")
    wm3r = nc.dram_tensor("wm3r", [128, 16], f32r, kind="ExternalInput")
    b1r = nc.dram_tensor("b1r", [128, 16], f32, kind="ExternalInput")
    b2r = nc.dram_tensor("b2r", [128, 16], f32, kind="ExternalInput")
    b3r = nc.dram_tensor("b3r", [128, 4], f32, kind="ExternalInput")
    bm1r = nc.dram_tensor("bm1r", [128, 16], f32, kind="ExternalInput")
    bm2r = nc.dram_tensor("bm2r", [128, 16], f32, kind="ExternalInput")
    bm3r = nc.dram_tensor("bm3r", [1, 1], f32, kind="ExternalInput")
    qT = nc.dram_tensor("qT", [1, B], f32, kind="ExternalOutput")

    with tile.TileContext(nc) as tc:
        with (
            tc.tile_pool(name="acts", bufs=1) as acts,
            tc.tile_pool(name="wts", bufs=6) as wts,
            tc.tile_pool(name="bias", bufs=1) as bias_pool,
            tc.tile_pool(name="psum", bufs=8, space="PSUM") as psum_pool,
        ):
            # --- bias / tiny-weight tiles ---
            def _load_const(dram, shape, name, dt=f32):
                t = bias_pool.tile(shape, dt, tag=name, name=name)
                nc.sync.dma_start(t[:, :], dram[:, :])
                return t

            tb1 = _load_const(b1r, [128, 16], "tb1")
            tb2 = _load_const(b2r, [128, 16], "tb2")
            tb3 = _load_const(b3r, [128, 4], "tb3")
            tbm1 = _load_const(bm1r, [128, 16], "tbm1")
            tbm2 = _load_const(bm2r, [128, 16], "tbm2")
            tbm3 = _load_const(bm3r, [1, 1], "tbm3")
            twm3 = _load_const(wm3r, [128, 16], "twm3", dt=f32r)

            # --- input activations, feature-major ---
            obs_tiles = []
            for k, kp in enumerate(_k_tiles(OBS_DIM)):
                t = acts.tile([128, B], f32r, tag="io", bufs=8, name=f"obs{k}")
                nc.sync.dma_start(t[:kp, :], obsT[k * 128 : k * 128 + kp, :])
                obs_tiles.append(t)
            act_tile = acts.tile([ACTION_DIM, B], f32r, tag="act", bufs=1, name="act")
            nc.sync.dma_start(act_tile[:, :], actT[:, :])

            def layer(parts, n_f, bias_tile, func, out_tag, out_bufs, out_name):
                """parts: list of (W_dram, x_tiles, k_sizes, row_base).
                n_f: number of 128-wide output feature tiles.
                Returns list of n_f SBUF tiles [128, B] holding out^T."""
                do_dma = mode in ("full", "dma")
                do_mm = mode in ("full", "pe")
                out_tiles = [
                    acts.tile([128, B], f32r, tag=out_tag, bufs=out_bufs,
                              name=f"{out_name}_{f}")
                    for f in range(n_f)
                ]
                total_k = sum(len(p[2]) for p in parts)
                shared_slab = None
                if mode == "pe":
                    shared_slab = wts.tile([128, FG * 128], f32r, tag="wshare",
                                           bufs=1, name=f"wshare_{out_name}")
                    nc.sync.dma_start(
                        shared_slab[:, :], parts[0][0][0:128, 0:FG * 128])
                for fg in range(n_f // FG):
                    ps = [
                        psum_pool.tile([128, BT], f32, tag="ps",
                                       name=f"ps_{out_name}_{fg}_{i}")
                        for i in range(FG * NB)
                    ]
                    step = 0
                    for W, x_tiles, k_sizes, row_base in parts:
                        for k, kp in enumerate(k_sizes):
                            if mode == "pe":
                                slab = shared_slab
                            else:
                                slab = wts.tile([128, FG * 128], f32r, tag="w",
                                                name=f"w_{out_name}_{fg}_{step}")
                                r0 = row_base + k * 128
                                c0 = fg * FG * 128
                                nc.sync.dma_start(
                                    slab[:kp, :], W[r0 : r0 + kp, c0 : c0 + FG * 128]
                                )
                            if do_mm:
                                for fi in range(FG):
                                    lhsT = slab[:kp, fi * 128 : (fi + 1) * 128]
                                    for b in range(NB):
                                        nc.tensor.matmul(
                                            ps[fi * NB + b][:, :],
                                            lhsT,
                                            x_tiles[k][:kp, b * BT : (b + 1) * BT],
                                            start=(step == 0),
                                            stop=(step == total_k - 1),
                                        )
                            step += 1
                    if do_mm:
                        for fi in range(FG):
                            f = fg * FG + fi
                            for b in range(NB):
                                nc.scalar.activation(
                                    out_tiles[f][:, b * BT : (b + 1) * BT],
                                    ps[fi * NB + b][:, :],
                                    func,
                                    bias=bias_tile[:, f : f + 1],
                                )
                return out_tiles

            for rep in range(repeats):
                sfx = f"r{rep}_" if repeats > 1 else ""
                h1 = layer([(W1, obs_tiles, _k_tiles(OBS_DIM), 0)],
                           HAN_HIDDEN // 128, tb1, Relu, "big", 32, sfx + "h1")
                h2 = layer([(W2, h1, _k_tiles(HAN_HIDDEN), 0)],
                           HAN_HIDDEN // 128, tb2, Relu, "big", 32, sfx + "h2")
                emb = layer([(W3, h2, _k_tiles(HAN_HIDDEN), 0)],
                            HAN_OUT // 128, tb3, Ident, "emb", 4, sfx + "emb")
                h3 = layer(
                    [(Wm1a, [act_tile], [ACTION_DIM], 0),
                     (Wm1e, emb, _k_tiles(HAN_OUT), 0)],
                    MLP_HIDDEN // 128, tbm1, Relu, "big", 32, sfx + "h3")
                h4 = layer([(Wm2, h3, _k_tiles(MLP_HIDDEN), 0)],
                           MLP_HIDDEN // 128, tbm2, Relu, "big", 32, sfx + "h4")

                # --- final layer: q^T[1, B] = Wm3.T @ h4^T + bm3 ---
                q_sbuf = acts.tile([1, B], f32, tag="q", bufs=2,
                                   name=sfx + "q_sbuf")
                n_k6 = MLP_HIDDEN // 128
                if mode != "dma":
                    for b in range(NB):
                        ps = psum_pool.tile([128, BT], f32, tag="ps",
                                            name=f"ps_{sfx}q_{b}")
                        for k in range(n_k6):
                            nc.tensor.matmul(
                                ps[:1, :],
                                twm3[:, k : k + 1],
                                h4[k][:, b * BT : (b + 1) * BT],
                                start=(k == 0),
                                stop=(k == n_k6 - 1),
                            )
                        nc.scalar.activation(
                            q_sbuf[:1, b * BT : (b + 1) * BT],
                            ps[:1, :],
                            Ident,
                            bias=tbm3[:1, :1],
                        )
                if rep == repeats - 1:
                    if mode == "dma":
                        nc.gpsimd.memset(q_sbuf[:1, :], 0.0)
                    nc.sync.dma_start(qT[:, :], q_sbuf[:1, :])

    _split_excess_waits(nc)
    return nc


def make_in_maps(inputs):
    """Host-side sharding + layout massaging. Returns list of 8 per-core maps."""
    obs = np.asarray(inputs["obs"], dtype=np.float32)
    action = np.asarray(inputs["action"], dtype=np.float32)
    Wm1 = np.asarray(inputs["Wm1"], dtype=np.float32)

    shared = {
        "W1": np.ascontiguousarray(inputs["W1"], dtype=np.float32),
        "W2": np.ascontiguousarray(inputs["W2"], dtype=np.float32),
        "W3": np.ascontiguousarray(inputs["W3"], dtype=np.float32),
        "Wm1a": np.ascontiguousarray(Wm1[:ACTION_DIM]),
        "Wm1e": np.ascontiguousarray(Wm1[ACTION_DIM:]),
        "Wm2": np.ascontiguousarray(inputs["Wm2"], dtype=np.float32),
        "wm3r": np.ascontiguousarray(
            np.asarray(inputs["Wm3"], dtype=np.float32).reshape(16, 128).T),
        "b1r": np.ascontiguousarray(
            np.asarray(inputs["b1"], dtype=np.float32).reshape(16, 128).T),
        "b2r": np.ascontiguousarray(
            np.asarray(inputs["b2"], dtype=np.float32).reshape(16, 128).T),
        "b3r": np.ascontiguousarray(
            np.asarray(inputs["b3"], dtype=np.float32).reshape(4, 128).T),
        "bm1r": np.ascontiguousarray(
            np.asarray(inputs["bm1"], dtype=np.float32).reshape(16, 128).T),
        "bm2r": np.ascontiguousarray(
            np.asarray(inputs["bm2"], dtype=np.float32).reshape(16, 128).T),
        "bm3r": np.asarray(inputs["bm3"], dtype=np.float32).reshape(1, 1),
    }
    in_maps = []
    for c in range(N_CORES):
        sl = slice(c * B, (c + 1) * B)
        m = dict(shared)
        m["obsT"] = np.ascontiguousarray(obs[sl].T)
        m["actT"] = np.ascontiguousarray(action[sl].T)
        in_maps.append(m)
    return in_maps


def run_sharded(inputs):
    """Build + run on 8 cores; returns per-core results list."""
    from concourse.bass_utils import run_bass_kernel_spmd

    nc = _build()
    in_maps = make_in_maps(inputs)
    res = run_bass_kernel_spmd(nc, in_maps, core_ids=list(range(N_CORES)))
    return res


def kernel(**inputs):
    res = run_sharded(inputs)
    q = np.empty((BATCH, 1), np.float32)
    for c in range(N_CORES):
        q[c * B : (c + 1) * B, 0] = res.results[c]["qT"][0]
    return q

